# revision 20
# baseline (speedup 1.0000x reference)
"""Single-head causal attention on 8 Trainium2 NeuronCores (Bass/Tile).

Problem: x [4, 4096, 1024] f32, Wq/Wk/Wv [64, 1024] f32 ->
         softmax(causal(q k^T * H^-0.5)) v   -> [4, 4096, 64] f32

Sharding: core = (batch b, parity p), b = core//2, p = core%2. Each core owns
the global 128-wide query tiles g = 2j+p (j=0..15) of its batch -- the parity
interleave balances causal work AND keeps the compiled graph identical across
all 8 cores (SPMD: one NEFF). All parity differences live in host-prepared
data, never in the graph:

  * x arrives as a per-core SHIFTED transpose xt [C, T] whose 128-col key
    blocks are: p=0 -> [zeros | x.T blocks 0..30], p=1 -> [x.T blocks 0..31].
    In this local key space both parities share identical causal geometry:
    local key block k' is fully visible to local query tile r of chunk ch
    (global q-tile g = 8ch+2r+p) iff k' <= 8ch+2r, diagonal (lower-tri mask)
    at k' = 8ch+2r+1, fully masked beyond -- parity-free.
  * The zero-pad block contributes exp(0)*128 = 128 to every softmax
    denominator of p=0 cores; a host-supplied per-core constant (dbias)
    subtracts it exactly before the reciprocal.
  * Queries live in the odd local key blocks (orig g = 2j+p <-> k' = 2j+1),
    so Q projection reads a strided view of the same resident xt -- no
    second copy of x is transferred.

Device pipeline (bf16 matmuls, f32 PSUM accumulation):
  1. xt fully SBUF-resident via 16 large DMAs (2-6 KB lines).
  2. Q^T projection with duplicated weights [Wq.T|Wq.T]; fused [K^T;V^T]
     projection; K^T lands on PSUM rows 64:128 and is repartitioned to rows
     0:64 via SBUF->SBUF DMA (GpSimd ring, off the main DMA stream); V^T ->
     V via PE transposes with a ones-column so the softmax denominator falls
     out of the AV matmul (row 64 of O^T).
  3. Per 512-wide q-chunk ch (extent 8ch+8 k-tiles, in pairs): S^T tiles
     [128k, 512q] on PE -> exp on ScalarE (scale=0.125 folded) -> lower-tri
     mask multiply on the diagonal blocks (DVE) -> AV accumulation. Fully
     masked left col-blocks are suffix-sliced out of S^T/exp/AV. Next
     phase's projection work is drip-fed between groups so ScalarE (the
     critical engine) never starves.
  4. Epilogue per chunk: PE transpose [65,128]->[128,65], subtract dbias,
     reciprocal, scale, DMA out.
"""
import os

import numpy as np
import ml_dtypes

import concourse.bass as bass
import concourse.mybir as mybir
import concourse.tile as tile
from concourse import bacc
from concourse.bass_utils import run_bass_kernel_spmd
from concourse.masks import make_identity

P = 128
B, T, C, H = 4, 4096, 1024, 64
TQ = T // 2          # queries per core
CH = 512             # q-chunk width
NCH = TQ // CH       # 4 q-chunks
CT = C // P          # 8 contraction tiles
TC = T // CH         # 8 t-chunks for K/V proj
NKT = T // P         # 32 k-tiles
GROUP = 2            # k-tiles per exp group
N_CORES = 8

F32 = mybir.dt.float32
BF16 = mybir.dt.bfloat16
Exp = mybir.ActivationFunctionType.Exp
MULT = mybir.AluOpType.mult
SUB = mybir.AluOpType.subtract

LAST_EXEC_TIME_NS = None
_COMPILED = None


def _build_graph():
    nc = bacc.Bacc("TRN2", target_bir_lowering=False, debug=False,
                   num_devices=N_CORES)
    xt = nc.dram_tensor("xt", [C, T], BF16, kind="ExternalInput").ap()
    wqq = nc.dram_tensor("wqq", [C, P], BF16, kind="ExternalInput").ap()
    wkv = nc.dram_tensor("wkv", [C, P], BF16, kind="ExternalInput").ap()
    mtri = nc.dram_tensor("mtri", [P, P], BF16, kind="ExternalInput").ap()
    dbias = nc.dram_tensor("dbias", [P, 1], F32, kind="ExternalInput").ap()
    y = nc.dram_tensor("y", [TQ, H], F32, kind="ExternalOutput").ap()

    xt_r = xt.rearrange("(co p) t -> p co t", p=P)     # [128, 8, 4096]
    wqq_r = wqq.rearrange("(co p) m -> p co m", p=P)   # [128, 8, 128]
    wkv_r = wkv.rearrange("(co p) m -> p co m", p=P)

    with tile.TileContext(nc) as tc:
        with (
            tc.tile_pool(name="const", bufs=1) as const,
            tc.tile_pool(name="ssb", bufs=3) as sspool,
            tc.tile_pool(name="epi", bufs=2) as epool,
            tc.tile_pool(name="pproj", bufs=1, space="PSUM") as ppool,
            tc.tile_pool(name="ps", bufs=2, space="PSUM") as spool,
            tc.tile_pool(name="po", bufs=2, space="PSUM") as opool,
            tc.tile_pool(name="pt", bufs=1, space="PSUM") as tpool,
        ):
            # ---- constants ----
            wqq_sb = const.tile([P, CT, P], BF16, name="wqq_sb")
            wkv_sb = const.tile([P, CT, P], BF16, name="wkv_sb")
            mask_sb = const.tile([P, P], BF16, name="mask_sb")
            dbias_sb = const.tile([P, 1], F32, name="dbias_sb")
            ident16 = const.tile([P, P], BF16, name="ident16")
            ident32 = const.tile([P, P], F32, name="ident32")
            scratch = const.tile([P, 1], F32, name="scratch")
            nc.gpsimd.dma_start(wqq_sb[:], wqq_r)
            nc.gpsimd.dma_start(wkv_sb[:], wkv_r)
            nc.gpsimd.dma_start(mask_sb[:], mtri)
            nc.gpsimd.dma_start(dbias_sb[:], dbias)
            # preload the exp table set immediately (scratch <- exp(0))
            nc.vector.memset(scratch[:], 0.0)
            nc.scalar.activation(scratch[:], scratch[:], Exp)
            make_identity(nc, ident16[:])
            make_identity(nc, ident32[:])

            # ---- resident x ----
            xt_sb = const.tile([P, CT, T], BF16, name="xt_sb")
            # odd local key blocks hold this core's query tokens
            xt_q = xt_sb.rearrange("p co (hb two q) -> p co hb two q",
                                   two=2, q=P)          # [128, 8, 16, 2, 128]

            # ---- persistent activations ----
            qt_sb = const.tile([P, TQ], BF16, name="qt_sb")      # Q^T dup rows
            kt_sb = const.tile([P, T], BF16, name="kt_sb")       # K^T top, zero bottom
            kstage = const.tile([P, T], BF16, name="kstage")     # K^T at rows 64:128
            vt_sb = const.tile([64, T], BF16, name="vt_sb")      # V^T
            v_sb = const.tile([P, NKT, H + 1], BF16, name="v_sb")  # V tiles + ones

            nc.gpsimd.memset(kt_sb[64:128, :], 0.0)
            nc.gpsimd.memset(v_sb[:, :, H:H + 1], 1.0)

            # ---- DMA schedule ----
            # One dma_start per 1024-col wave: a single InstDMACopy is split
            # across all 16 SDMA engines, and HWDGE executes FIFO per ring --
            # few BIG transfers (2 MB, 2KB lines) run at ~340 GB/s while many
            # small ones serialize at descriptor rate. Wave w feeds phase w.
            # Per-(c-tile, wave) DMAs: contiguous 2KB-line transfers whose
            # flat address ranges stay precise for Tile's dependency tracker
            # (multi-co strided writes poison every later reader). Split
            # round-robin across BOTH HWDGE rings (sync + scalar) -- each ring
            # executes FIFO, the 16 SDMA engines interleave rings per packet.
            # all x waves on the sync ring ONLY: dma_start issues on the
            # scalar ring backpressure the ScalarE sequencer and stall the
            # exp stream behind the whole x transfer
            for w in range(NCH):
                for c in range(CT):
                    nc.sync.dma_start(xt_sb[:, c, bass.ts(w, 2 * CH)],
                                      xt_r[:, c, bass.ts(w, 2 * CH)])

            # ---- projection work units (drip-fed between attention groups) --
            def q_proj_units(qc):
                ps = ppool.tile([P, CH], F32, tag="ps_proj")
                for c in range(CT):
                    yield lambda c=c, ps=ps: nc.tensor.matmul(
                        ps[:], lhsT=wqq_sb[:, c, :],
                        rhs=xt_q[:, c, bass.ts(qc, 4), 1, :],
                        start=(c == 0), stop=(c == CT - 1))
                yield lambda ps=ps: nc.vector.tensor_copy(
                    qt_sb[:, bass.ts(qc, CH)], ps[:])

            def kv_proj_units(t_i):
                ps = ppool.tile([P, CH], F32, tag="ps_proj")
                for c in range(CT):
                    yield lambda c=c, ps=ps: nc.tensor.matmul(
                        ps[:], lhsT=wkv_sb[:, c, :],
                        rhs=xt_sb[:, c, bass.ts(t_i, CH)],
                        start=(c == 0), stop=(c == CT - 1))

                def evac_k(ps=ps):
                    nc.vector.tensor_copy(kstage[64:128, bass.ts(t_i, CH)],
                                          ps[64:128, :])
                    nc.gpsimd.dma_start(kt_sb[0:64, bass.ts(t_i, CH)],
                                        kstage[64:128, bass.ts(t_i, CH)])
                yield evac_k
                yield lambda ps=ps: nc.vector.tensor_copy(
                    vt_sb[:, bass.ts(t_i, CH)], ps[0:64, :])
                for j in range(CH // P):
                    def vtile(j=j):
                        kt = t_i * (CH // P) + j
                        pt = tpool.tile([P, P], BF16, tag="tr")
                        nc.tensor.transpose(pt[:, 0:64], vt_sb[:, bass.ts(kt, P)],
                                            ident16[0:64, 0:64])
                        nc.vector.tensor_copy(v_sb[:, kt, 0:H], pt[:, 0:64])
                    yield vtile

            def epilogue_units(ch, po):
                # po is fully accumulated; evacuate it promptly (releases the
                # single po slot), then normalize+store subtile by subtile.
                osb = epool.tile([H + 1, CH], F32, tag="osb")
                yield lambda: nc.vector.tensor_copy(osb[:], po[0:H + 1, :])
                for s in range(CH // P):
                    def sub(s=s):
                        pt2 = tpool.tile([P, P], F32, tag="tr")
                        nc.tensor.transpose(pt2[:, 0:H + 1], osb[:, bass.ts(s, P)],
                                            ident32[0:H + 1, 0:H + 1])
                        den = epool.tile([P, 1], F32, tag="den")
                        nc.vector.tensor_tensor(den[:], pt2[:, H:H + 1],
                                                dbias_sb[:], SUB)
                        rec = epool.tile([P, 1], F32, tag="rec")
                        nc.vector.reciprocal(rec[:], den[:])
                        ot = epool.tile([P, H], F32, tag="ot")
                        nc.vector.tensor_scalar_mul(ot[:], pt2[:, 0:H], rec[:])
                        nc.gpsimd.dma_start(y[bass.ds(ch * CH + s * P, P), :], ot[:])
                    yield sub

            def phase_units(phase):
                # Q first: it gates the next chunk's very first S^T; the KV
                # chunks' V-tiles are only needed by progressively later AVs
                yield from q_proj_units(phase)
                yield from kv_proj_units(2 * phase)
                yield from kv_proj_units(2 * phase + 1)

            # ---- attention: flat one-group-lookahead software pipeline ----
            # PE is in-order, and AV(g) must wait exp(g); emitting S^T(g+1)
            # BEFORE AV(g) lets the next group's scores (into the other
            # double-buffered PSUM slot) compute while exp(g) runs, so the
            # ScalarE exp stream runs back-to-back.
            def n_groups_of(ch):
                return (8 * ch + 8) // GROUP

            def emit_st(ch, g):
                # left col-blocks with k'-8ch-2r >= 2 are fully masked:
                # suffix-slice them out of S^T, exp and AV. Both k-tiles of
                # the pair share r0 = max(0, g - 4ch).
                r0 = max(0, g - 4 * ch)
                ps_s = spool.tile([P, GROUP * CH], F32, name="ps_s")
                s_sb = sspool.tile([P, GROUP * CH], BF16, tag="s_sb")
                ps_v = ps_s.rearrange("p (j w) -> p j w", j=GROUP)
                s_v = s_sb.rearrange("p (j w) -> p j w", j=GROUP)
                for j in range(GROUP):
                    kt = g * GROUP + j
                    nc.tensor.matmul(
                        ps_v[:, j, r0 * P:CH],
                        lhsT=kt_sb[:, bass.ts(kt, P)],
                        rhs=qt_sb[:, ch * CH + r0 * P: (ch + 1) * CH],
                        start=True, stop=True)
                return ps_v, s_v, r0

            # Chunk-PAIR interleave: part A walks (ch0, ch1) k-pair-major,
            # part B walks (ch2, ch3). Each step issues ~2 exp tiles, so the
            # ScalarE stream is dense while projection/epilogue work drips in
            # between; only 2 po accumulators are ever alive. Within a part
            # the later chunk lags 2 steps so its Q projection (a later DMA
            # wave) has time to land.
            def pair_items(lo, hi):
                n_lo, n_hi = n_groups_of(lo), n_groups_of(hi)
                items = []
                for jj in range(n_hi + 2):
                    if jj < n_lo:
                        items.append((lo, jj))
                    if jj >= 2:
                        items.append((hi, jj - 2))
                return items

            flat = pair_items(0, 1) + pair_items(2, 3)
            last_idx = {ch: max(i for i, it in enumerate(flat) if it[0] == ch)
                        for ch in range(NCH)}

            # Projection feed, wave-ordered. Units are drained (a) on demand
            # before an S^T that depends on them is emitted (PE is in-order,
            # so emitting a dependent S^T first would deadlock against units
            # emitted behind it), and (b) at a steady trickle for pacing.
            from collections import deque

            class Feed:
                def __init__(self):
                    self.q = deque()
                    self.done = set()

                def add(self, tag, units):
                    units = list(units)
                    for k, u in enumerate(units):
                        self.q.append((tag if k == len(units) - 1 else None, u))

                def prepend(self, units):
                    for u in reversed(list(units)):
                        self.q.appendleft((None, u))

                def emit_one(self):
                    if not self.q:
                        return False
                    tag, u = self.q.popleft()
                    u()
                    if tag is not None:
                        self.done.add(tag)
                    return True

                def require(self, tag):
                    while tag not in self.done and self.q:
                        self.emit_one()

                def drain(self):
                    while self.emit_one():
                        pass

            feed = Feed()
            feed.add(("KV", 1), kv_proj_units(1))
            feed.add(("Q", 1), q_proj_units(1))
            feed.add(("KV", 2), kv_proj_units(2))
            feed.add(("KV", 3), kv_proj_units(3))
            feed.add(("Q", 2), q_proj_units(2))
            feed.add(("KV", 4), kv_proj_units(4))
            feed.add(("KV", 5), kv_proj_units(5))
            feed.add(("Q", 3), q_proj_units(3))
            feed.add(("KV", 6), kv_proj_units(6))
            feed.add(("KV", 7), kv_proj_units(7))
            feed.done.add(("Q", 0))
            feed.done.add(("KV", 0))
            per_step = 3

            # pre-work: Q0 + KV0 cover the first two steps (kt 0..3)
            for u in kv_proj_units(0):
                u()
            for u in q_proj_units(0):
                u()

            def emit_st_safe(ch, g):
                feed.require(("Q", ch))
                feed.require(("KV", (2 * g + 1) // 4))
                return emit_st(ch, g)

            pending = {flat[0]: emit_st_safe(*flat[0])}
            po = {}
            for i, (ch, g) in enumerate(flat):
                ext = 8 * ch + 8
                if g == 0:
                    po[ch] = opool.tile([P, CH], F32, name=f"po{ch}", tag="po")
                if i + 1 < len(flat):
                    pending[flat[i + 1]] = emit_st_safe(*flat[i + 1])
                ps_v, s_v, r0 = pending.pop((ch, g))
                nc.scalar.activation(s_v[:, :, r0 * P:CH],
                                     ps_v[:, :, r0 * P:CH], Exp, scale=0.125)
                for j in range(GROUP):
                    kt = g * GROUP + j
                    for r in range(r0, 4):
                        if kt - 8 * ch - 2 * r == 1:  # diagonal block
                            blk = s_v[:, j, r * P:(r + 1) * P]
                            nc.vector.tensor_tensor(blk, blk, mask_sb[:], MULT)
                for j in range(GROUP):
                    kt = g * GROUP + j
                    nc.tensor.matmul(po[ch][0:H + 1, r0 * P:CH],
                                     lhsT=v_sb[:, kt, :],
                                     rhs=s_v[:, j, r0 * P:CH],
                                     start=(kt == 0), stop=(kt == ext - 1))
                for _ in range(per_step):
                    feed.emit_one()
                if i == last_idx[ch]:
                    # epilogue: evacuate po now (releases its slot); the
                    # per-subtile normalize/store units join the feed queue
                    epi = epilogue_units(ch, po.pop(ch))
                    next(epi)()
                    if i + 1 < len(flat):
                        feed.prepend(epi)
                    else:
                        feed.drain()
                        for u in epi:
                            u()

    nc.compile()
    return nc


def _shard_inputs(x, Wq, Wk, Wv):
    bf = ml_dtypes.bfloat16
    tri = np.tril(np.ones((P, P), dtype=np.float32)).T  # [kk,qq]=1 iff kk<=qq
    wqq = np.concatenate([Wq.T, Wq.T], axis=1).astype(bf)
    wkv = np.concatenate([Wv.T, Wk.T], axis=1).astype(bf)
    mtri = tri.astype(bf)
    in_maps = []
    for core in range(N_CORES):
        b, p = core // 2, core % 2
        if p == 0:
            # [zeros | blocks 0..30]
            xt_full = np.concatenate(
                [np.zeros((P, C), dtype=np.float32), x[b][:T - P]], axis=0).T
        else:
            xt_full = x[b].T
        xt_core = np.ascontiguousarray(xt_full).astype(bf)
        db = np.full((P, 1), 128.0 if p == 0 else 0.0, dtype=np.float32)
        in_maps.append({"xt": xt_core, "wqq": wqq, "wkv": wkv,
                        "mtri": mtri, "dbias": db})
    return in_maps


def _unshard(results):
    y = np.zeros((B, T, H), dtype=np.float32)
    for core in range(N_CORES):
        b, p = core // 2, core % 2
        yc = results[core]["y"]
        for j in range(16):
            g = 2 * j + p
            y[b, P * g:P * g + P] = yc[P * j:P * j + P]
    return y


def kernel(x, Wq, Wk, Wv):
    global LAST_EXEC_TIME_NS, _COMPILED
    x = np.asarray(x, dtype=np.float32)
    Wq = np.asarray(Wq, dtype=np.float32)
    Wk = np.asarray(Wk, dtype=np.float32)
    Wv = np.asarray(Wv, dtype=np.float32)

    if _COMPILED is None:
        _COMPILED = _build_graph()
    nc = _COMPILED

    in_maps = _shard_inputs(x, Wq, Wk, Wv)
    kwargs = {}
    if os.environ.get("ATTN_TRACE"):
        kwargs["trace"] = True
        if os.environ.get("ATTN_TRACE_DIR"):
            kwargs["tmpdir"] = os.environ["ATTN_TRACE_DIR"]
    res = run_bass_kernel_spmd(nc, in_maps, core_ids=list(range(N_CORES)), **kwargs)
    LAST_EXEC_TIME_NS = res.exec_time_ns
    return _unshard(res.results)
